# revision 33
# baseline (speedup 1.0000x reference)
"""Cosine multi-head attention (h=1) Trainium2 kernel.

Math (reference):
    context = query @ Wq.T + bq                  [B, S, HD]
    ctx     = context * weight_tensor[0]         (elementwise over HD)
    ctx_n   = ctx / max(||ctx||_2, eps)          (normalize over HD)
    scores  = ctx_n @ ctx_n.T                    [B, S, S]
    out     = softmax(scores, axis=-1)

Device strategy (8 cores, SPMD):
    core c handles batch b = c//2, row-half h = c%2.  The host rotates the
    batch's rows so each core's own 2048 rows come first, transposes to
    qT [D, S] bf16, and folds weight_tensor into Wq: M = diag(w) @ Wq
    (bf16), c0 = w * bq.  HD=120 is zero-padded to 128 on the host so every
    matmul runs K=M=128.

    On device (single-term bf16 matmuls; the 2e-2 harness gate leaves ~2.5x
    margin over the resulting ~8e-3 error):
      CT[hd, s] = sum_d M[hd, d] qT[d, s]      (PE, PSUM [128, 2048] per half)
      ctsq = (CT + c0)^2 -> bf16               (ACT Square, bias is free)
      ct   = CT + c0                           (DVE, frees the PSUM bank set)
      n2   = ones^T @ ctsq                     (PE broadcast-sum over HD)
      inv  = 1/sqrt(|n2 + 1e-20|)              (ACT Abs_reciprocal_sqrt)
      cn   = ct * inv -> bf16                  (DVE)
      per 128-row chunk i of the first 2048 rows:
         R = cn[:, i-chunk].T @ cn             (PE, PSUM [128, 2048] x2)
         E = exp(R) -> bf16 with fused row-sum (ACT accum_out)
         out_rows = E * (1/rowsum) -> bf16     (DVE 4x packed)
    Softmax needs no max-subtraction: scores are cosines in [-1, 1].

    Scheduling notes (in-order engine queues; deps are tile-granular):
    - cn/ct/ctsq/inv are per-half tiles so the first gram units depend only
      on half A.
    - The PE queue is warmup, ctA, ctB(c0-3), normA, ctB(c4-7), normB,
      gram...; zero-matmul warmup during the input DMA initializes the k=3
      accumulator region (live code) and holds the PE HAM clock at 2.4 GHz.
    - All four inv/sq ACT instructions precede every softmax exp: exactly
      two activation-table loads, none mid-stream.
    Output is written bf16 (absmax metric tolerates ~2e-3); host upcasts.
    Output columns of h=1 cores are rotated by 2048; the host gather undoes it.
"""

import numpy as np
from contextlib import ExitStack

B, S, D, HD = 4, 4096, 1024, 120
HDP = 128      # HD zero-padded so K=M=128 in every matmul
ROWS = S // 2  # rows of the score matrix each core produces
N_CORES = 8

_NC_CACHE = {}


def _build_nc():
    import concourse.bacc as bacc
    import concourse.tile as tile
    from concourse import mybir

    f32 = mybir.dt.float32
    bf16 = mybir.dt.bfloat16
    AF = mybir.ActivationFunctionType
    nc = bacc.Bacc("TRN2", target_bir_lowering=False, debug=False,
                   num_devices=N_CORES)

    q_in = nc.declare_dram_parameter("q_in", [D, S], bf16, isOutput=False)
    # mt rows padded to 256 cols so every DMA run is 512 B (line rate)
    mt = nc.declare_dram_parameter("mt", [D, 2 * HDP], bf16, isOutput=False)
    c0 = nc.declare_dram_parameter("c0", [HDP, 1], f32, isOutput=False)
    out = nc.declare_dram_parameter("out", [ROWS, S], bf16, isOutput=True)

    DC = D // 128   # 8 contraction chunks

    with ExitStack() as ctx:
        tc = ctx.enter_context(tile.TileContext(nc))
        singles = ctx.enter_context(tc.tile_pool(name="singles", bufs=1))
        qpool = ctx.enter_context(tc.tile_pool(name="qpool", bufs=1))
        work = ctx.enter_context(tc.tile_pool(name="work", bufs=1))
        epool = ctx.enter_context(tc.tile_pool(name="epool", bufs=3))
        spool = ctx.enter_context(tc.tile_pool(name="spool", bufs=4))
        ps = ctx.enter_context(tc.tile_pool(name="ps", bufs=2, space="PSUM"))

        # row = c*128 + p, col = h*2048 + j
        q_r = q_in.rearrange("(c p) (h j) -> h p c j", c=DC, p=128, h=2)
        ones_sq = singles.tile([HDP, HDP], bf16, tag="ones_sq")
        nc.vector.memset(ones_sq[:], 1.0)
        eps_sb = singles.tile([HDP, 1], f32, tag="eps")
        nc.vector.memset(eps_sb[:], 1e-20)
        zeros_w = singles.tile([128, 512], bf16, tag="zeros_w")
        nc.vector.memset(zeros_w[:], 0.0)

        # DMA queue order: qA0, mt, qA1, c0, qB0, qB1, qB2.  qA0 leads so
        # the first real matmuls start as early as possible (mt is only
        # needed at the same moment and is 4x smaller); half B arrives as
        # 2/1.5/0.5 MB pieces so the last matmuls chase a short final
        # transfer.  c0 is only needed by chain_a (late).
        mt_sb = singles.tile([128, DC, 2 * HDP], bf16, tag="mt")
        c0_sb = singles.tile([HDP, 1], f32, tag="c0")
        qsplit = {0: [(0, 4), (4, 8)], 1: [(0, 4), (4, 7), (7, 8)]}
        qtiles = {}
        for half in (0, 1):
            for si, (ca, cb) in enumerate(qsplit[half]):
                qt = qpool.tile([128, cb - ca, 2048], bf16,
                                tag=f"q{half}_{ca}", name=f"q{half}_{ca}")
                nc.sync.dma_start(out=qt[:], in_=q_r[half, :, ca:cb, :])
                for c in range(ca, cb):
                    qtiles[(half, c)] = (qt, c - ca)
                if half == 0 and si == 0:
                    nc.sync.dma_start(
                        out=mt_sb[:],
                        in_=mt.rearrange("(c p) h -> p c h", p=128))
                if half == 0 and si == 1:
                    nc.sync.dma_start(out=c0_sb[:], in_=c0[:])

        ct_ps = {}
        ctsq = {}
        ct_f = {}
        inv = {}
        cn = {}
        n_ps = {}
        for half in (0, 1):
            ct_ps[half] = ps.tile([HDP, 2048], f32, tag="ps4",
                                  name=f"ct_ps{half}")
            n_ps[half] = ps.tile([HDP, 2048], f32, tag="ps4",
                                 name=f"n_ps{half}")
        for half in (0, 1):
            ctsq[half] = work.tile([HDP, 2048], bf16, tag=f"ctsq{half}",
                                   name=f"ctsq{half}")
            ct_f[half] = work.tile([HDP, 2048], f32, tag=f"ct{half}",
                                   name=f"ct{half}")
            inv[half] = work.tile([HDP, 2048], f32, tag=f"inv{half}",
                                  name=f"inv{half}")
            cn[half] = work.tile([HDP, 2048], bf16, tag=f"cn{half}",
                                 name=f"cn{half}")

        NWARM = 22

        def phase1_mms(half, cs):
            for c in cs:
                qt, ci = qtiles[(half, c)]
                for k in range(4):
                    warm_cont = (half == 0 and k == 3)
                    nc.tensor.matmul(
                        ct_ps[half][:, k * 512:(k + 1) * 512],
                        lhsT=mt_sb[:, c, :HDP],
                        rhs=qt[:, ci, k * 512:(k + 1) * 512],
                        start=(c == 0 and not warm_cont),
                        stop=(c == DC - 1))

        def norm_mms(half):
            for k in range(4):
                nc.tensor.matmul(n_ps[half][:, k * 512:(k + 1) * 512],
                                 lhsT=ones_sq[:],
                                 rhs=ctsq[half][:, k * 512:(k + 1) * 512],
                                 start=True, stop=True)

        def chain_a(half):
            # ACT square (bias folded) + DVE copy; both read ct_ps
            nc.scalar.activation(out=ctsq[half][:], in_=ct_ps[half][:],
                                 func=AF.Square, bias=c0_sb[:])
            nc.vector.tensor_scalar_add(ct_f[half][:], ct_ps[half][:],
                                        c0_sb[:])

        def chain_b(half):
            nc.scalar.activation(out=inv[half][:], in_=n_ps[half][:],
                                 func=AF.Abs_reciprocal_sqrt, bias=eps_sb[:])
            nc.vector.tensor_mul(cn[half][:], ct_f[half][:], inv[half][:])

        # PE warm-up while the first q DMA is in flight (see module doc)
        for r in range(NWARM):
            nc.tensor.matmul(ct_ps[0][:, 3 * 512:4 * 512],
                             lhsT=ones_sq[:], rhs=zeros_w[:],
                             start=(r == 0), stop=False)

        # chain_a(1) precedes chain_b(0) so the B half's ACT square is not
        # queued behind invA on the in-order ACT engine
        phase1_mms(0, range(0, 8))
        chain_a(0)
        phase1_mms(1, range(0, 8))
        norm_mms(0)
        chain_a(1)
        chain_b(0)
        norm_mms(1)
        chain_b(1)

        # --- phase 3: gram + softmax; pairs of 128-row chunks share an
        #     output tile so DMA-out goes in 2 MB transfers ---
        NCHUNK = ROWS // 128
        for i in range(NCHUNK):
            ic = i % 2
            if ic == 0:
                e2 = epool.tile([128, 2, S], bf16, tag="e", name=f"e{i}")
                sums = spool.tile([128, 4], f32, tag="sums", name=f"sums{i}")
            hi_i = cn[0][:, i * 128:(i + 1) * 128]
            for jg in range(2):
                r_ps = ps.tile([128, 2048], f32, tag="ps4",
                               name=f"r_ps{i}_{jg}")
                for k in range(4):
                    nc.tensor.matmul(r_ps[:, k * 512:(k + 1) * 512],
                                     lhsT=hi_i,
                                     rhs=cn[jg][:, k * 512:(k + 1) * 512],
                                     start=True, stop=True)
                nc.scalar.activation(
                    out=e2[:, ic, jg * 2048:(jg + 1) * 2048],
                    in_=r_ps[:],
                    func=AF.Exp,
                    accum_out=sums[:, 2 * ic + jg:2 * ic + jg + 1],
                )
            tot = spool.tile([128, 1], f32, tag="tot", name=f"tot{i}")
            nc.vector.tensor_add(tot[:], sums[:, 2 * ic:2 * ic + 1],
                                 sums[:, 2 * ic + 1:2 * ic + 2])
            rec = spool.tile([128, 1], f32, tag="rec", name=f"rec{i}")
            nc.vector.reciprocal(rec[:], tot[:])
            if i >= NCHUNK - 2:
                # drain the tail in half-chunk scale+DMA pairs so the first
                # DMA overlaps the second scale (shorter critical path)
                for jg in range(2):
                    sl = slice(jg * 2048, (jg + 1) * 2048)
                    nc.vector.tensor_scalar_mul(e2[:, ic, sl],
                                                e2[:, ic, sl], rec[:])
                    nc.sync.dma_start(
                        out=out[i * 128:(i + 1) * 128, sl],
                        in_=e2[:, ic, sl])
                continue
            nc.vector.tensor_scalar_mul(e2[:, ic, :], e2[:, ic, :], rec[:])
            if ic == 1:
                nc.sync.dma_start(
                    out=out[(i - 1) * 128:(i + 1) * 128, :].rearrange(
                        "(c p) s -> p c s", p=128),
                    in_=e2[:],
                )

    nc.compile()
    return nc


def _get_nc():
    if "nc" not in _NC_CACHE:
        _NC_CACHE["nc"] = _build_nc()
    return _NC_CACHE["nc"]


def _make_in_maps(inputs):
    import ml_dtypes
    query = np.asarray(inputs["query"], dtype=np.float32)
    Wq = np.asarray(inputs["Wq"], dtype=np.float32)
    bq = np.asarray(inputs["bq"], dtype=np.float32)
    w = np.asarray(inputs["weight_tensor"], dtype=np.float32)

    w0 = w.reshape(-1)[:HD]
    mt_np = np.zeros((D, 2 * HDP), dtype=ml_dtypes.bfloat16)
    mt_np[:, :HD] = (w0[:, None] * Wq).T.astype(ml_dtypes.bfloat16)  # [D,HDP]
    c0_np = np.zeros((HDP, 1), dtype=np.float32)
    c0_np[:HD, 0] = w0 * bq

    in_maps = []
    for c in range(N_CORES):
        b, h = c // 2, c % 2
        qb = query[b]
        if h:
            qb = np.concatenate([qb[ROWS:], qb[:ROWS]], axis=0)
        q_np = np.ascontiguousarray(qb.T.astype(ml_dtypes.bfloat16))  # [D,S]
        in_maps.append({"q_in": q_np, "mt": mt_np, "c0": c0_np})
    return in_maps


def _gather(results):
    full = np.empty((B, S, S), dtype=np.float32)
    for c in range(N_CORES):
        b, h = c // 2, c % 2
        r = results[c]["out"]  # bf16 [ROWS, S]; assignment upcasts
        if h == 0:
            full[b, :ROWS] = r
        else:
            full[b, ROWS:, ROWS:] = r[:, :ROWS]
            full[b, ROWS:, :ROWS] = r[:, ROWS:]
    return full


def kernel(**inputs):
    from concourse.bass_utils import run_bass_kernel_spmd

    in_maps = _make_in_maps(inputs)
    nc = _get_nc()
    res = run_bass_kernel_spmd(nc, in_maps, list(range(N_CORES))).results
    return _gather(res)


def _register_ntff_hook():
    """Register the axon NTFF profile hook that the agent image's antenv
    package lacks (see trn_boot.py) so trace=True yields exec_time_ns."""
    import sys
    import types
    try:
        import antenv.axon_hooks  # noqa: F401
        return True
    except ImportError:
        pass
    try:
        from trn_agent_boot.trn_boot import _ntff_profile_via_ctypes
        hook = _ntff_profile_via_ctypes("/opt/axon/libaxon_pjrt.so")
    except Exception:
        return False
    if hook is None:
        return False
    mod = types.ModuleType("antenv.axon_hooks")
    mod._hook = hook
    mod.get_axon_ntff_profile_hook = lambda: mod._hook
    mod.set_axon_ntff_profile_hook = lambda h: setattr(mod, "_hook", h)
    sys.modules["antenv.axon_hooks"] = mod
    import antenv
    antenv.axon_hooks = mod
    return True


def profile_once(inputs, trace_cores=None):
    """Re-run the kernel with NTFF profiling; returns max exec_time_ns."""
    import tempfile
    import concourse.bass_utils as bu

    _register_ntff_hook()
    # avoid the cloud artifact upload inside the trace path
    bu.upload_artifacts = lambda tmpdir: tmpdir

    in_maps = _make_in_maps(inputs)
    nc = _get_nc()
    tmpdir = tempfile.mkdtemp(prefix="ntff_")
    r = bu.run_bass_kernel_spmd(nc, in_maps, list(range(N_CORES)),
                                trace=True, trace_cores=trace_cores,
                                tmpdir=tmpdir)
    print(f"trace dir: {tmpdir}")
    if r.exec_time_ns is not None:
        print(f"mean exec: {r.mean_exec_time_ns} ns, "
              f"max core: {r.max_exec_time_core_id}")
    return r.exec_time_ns
